# revision 4
# baseline (speedup 1.0000x reference)
"""Trainium2 Bass kernel for nn_BehaviorVelocity (self-contained).

Strategy: 8 cores = (batch b, H-half). Per core: 3 independent 128-row bands
(102 unique rows + 13-row halos); all 17 stencil steps run without cross-band
communication. H(partition)-shifts via PE shift-matrix matmuls into PSUM;
W-shifts via free-dim AP offsets with zeroed guard columns. Direction-bucket
computed via 4 threshold compares (exactly equivalent to the arccos form).
Payload channels (2..17) in bf16; mask-critical state (empty, wall, velocity)
in f32.
"""
import numpy as np
import ml_dtypes
import concourse.bass as bass
import concourse.mybir as mybir
import concourse.tile as tile
from concourse.bass_utils import run_bass_kernel_spmd

F32 = mybir.dt.float32
BF16 = mybir.dt.bfloat16
ALU = mybir.AluOpType

DIRS = [(-1, 0), (-1, 1), (0, 1), (1, 1), (1, 0), (1, -1), (0, -1), (-1, -1)]
B, C, H, W = 4, 20, 512, 512   # world shape; C = 14 elements + 6
NEV = 18                        # world channels that evolve (ch 18,19 are overwritten)
NP = 16                         # payload channels 2..17
HALO = 13
UNIQ = 128 - 2 * HALO           # 102 unique rows per band
WP = W + 2                      # guard cols at 0 and 513
COS_B = [float(np.float32(np.cos((2 * k + 1) * np.pi / 8))) for k in range(4)]

_CACHE = {}


def _fix_sync_overflow(nc, max_waits=1, max_updates=1):
    """Split multi-wait/update instructions into single-sync NoOps (walrus limit)."""
    n_fix = 0
    for fn in nc.m.functions:
        for bb in fn.blocks:
            old = list(bb.instructions)
            new = []
            changed = False
            for ins in old:
                si = ins.sync_info
                waits = list(si.on_wait) if si is not None and si.on_wait else []
                ups = list(si.on_update) if si is not None and si.on_update else []
                pre, post = [], []
                if len(waits) > max_waits:
                    for i, w in enumerate(waits[:len(waits) - max_waits]):
                        pre.append(mybir.InstNoOp(
                            name=f"{ins.name}_w{i}", engine=ins.engine,
                            sync_info=mybir.SyncInfo(on_wait=[w], on_update=[])))
                    waits = waits[len(waits) - max_waits:]
                    n_fix += 1
                if len(ups) > max_updates:
                    for i, u in enumerate(ups[max_updates:]):
                        post.append(mybir.InstNoOp(
                            name=f"{ins.name}_u{i}", engine=ins.engine,
                            sync_info=mybir.SyncInfo(on_wait=[], on_update=[u])))
                    ups = ups[:max_updates]
                    n_fix += 1
                if pre or post:
                    ins.sync_info = mybir.SyncInfo(on_wait=waits, on_update=ups)
                    changed = True
                new.extend(pre)
                new.append(ins)
                new.extend(post)
            if changed:
                bb.instructions = new
    return n_fix


def _build_nc():
    nc = bass.Bass()
    inpf = nc.declare_dram_parameter("inpf", [3, 128, 4, WP], F32, isOutput=False)
    inpp = nc.declare_dram_parameter("inpp", [3, 128, NP, WP], BF16, isOutput=False)
    smf_d = nc.declare_dram_parameter("smf", [128, 3, 128], F32, isOutput=False)
    smb_d = nc.declare_dram_parameter("smb", [128, 3, 128], BF16, isOutput=False)
    outf = nc.declare_dram_parameter("outf", [3, UNIQ, 4, W], F32, isOutput=True)
    outp = nc.declare_dram_parameter("outp", [3, UNIQ, NP, W], BF16, isOutput=True)

    sem = nc.alloc_semaphore()
    sv = 0

    def sb(name, shape, dtype):
        return nc.alloc_sbuf_tensor(name, list(shape), dtype).ap()

    stf = [sb(f"stf{j}", [128, 4, WP], F32) for j in range(3)]     # empty, wall, v0, v1
    stp = [sb(f"stp{j}", [128, NP, WP], BF16) for j in range(3)]   # payload ch 2..17
    vd = [sb(f"vd{j}", [128, 2, WP], F32) for j in range(3)]
    bkt = [sb(f"bkt{j}", [128, WP], F32) for j in range(3)]
    sa = [sb(f"sa{j}", [128, WP], BF16) for j in range(3)]
    smf = sb("smf_t", [128, 3, 128], F32)   # [:,0]=I, [:,1]=up (out[p]=in[p+1]), [:,2]=dn
    smb = sb("smb_t", [128, 3, 128], BF16)

    for j in range(3):
        nc.sync.dma_start(stf[j][:], inpf[j]).then_inc(sem, 16); sv += 16
        nc.sync.dma_start(stp[j][:], inpp[j]).then_inc(sem, 16); sv += 16
    nc.sync.dma_start(smf[:], smf_d[:]).then_inc(sem, 16); sv += 16
    nc.sync.dma_start(smb[:], smb_d[:]).then_inc(sem, 16); sv += 16
    for eng in nc.engines.values():
        eng.wait_ge(sem, sv)

    # shift index for "out[p] = in[p+dy]": dy=+1 -> 1 (up), dy=-1 -> 2 (dn), 0 -> 0
    def sh_idx(dy):
        return 0 if dy == 0 else (1 if dy == 1 else 2)

    with tile.TileContext(nc) as tc:
        with tc.tile_pool(name="mk", bufs=2) as mk, \
             tc.tile_pool(name="bk", bufs=1) as bk, \
             tc.tile_pool(name="big", bufs=2) as big, \
             tc.tile_pool(name="cv", bufs=1) as cv, \
             tc.tile_pool(name="ps", bufs=2, space="PSUM") as psp, \
             tc.tile_pool(name="pm", bufs=2, space="PSUM") as pmp:

            # zero mask guard cols once (sa persistent; guards must stay 0)
            for j in range(3):
                nc.vector.memset(sa[j][:], 0.0)

            for it in range(2):
                # ---- vd init / refresh: vd = velocity (stf ch 2,3) ----
                for j in range(3):
                    if it == 0:
                        nc.vector.tensor_copy(out=vd[j][:], in_=stf[j][:, 2:4, :])
                    else:
                        nc.vector.tensor_copy(out=stf[j][:, 2:4, :], in_=vd[j][:])

                # ---- bucket per band ----
                for j in range(3):
                    v0 = stf[j][:, 2, :]
                    v1 = stf[j][:, 3, :]
                    m2 = bk.tile([128, WP], F32, tag="m2")
                    tmp = bk.tile([128, WP], F32, tag="tmp")
                    nc.vector.tensor_tensor(out=m2[:], in0=v0, in1=v0, op=ALU.mult)
                    nc.vector.tensor_tensor(out=tmp[:], in0=v1, in1=v1, op=ALU.mult)
                    nc.vector.tensor_tensor(out=m2[:], in0=m2[:], in1=tmp[:], op=ALU.add)
                    mag = bk.tile([128, WP], F32, tag="mag")
                    nc.scalar.sqrt(out=mag[:], in_=m2[:])
                    # one Newton step: mag' = 0.5*(mag + m2/mag), then +0.001
                    rden = bk.tile([128, WP], F32, tag="rden")
                    nc.vector.tensor_scalar_add(out=rden[:], in0=mag[:], scalar1=1e-30)
                    rcp = bk.tile([128, WP], F32, tag="rcp")
                    nc.vector.reciprocal(out=rcp[:], in_=rden[:])
                    nc.vector.tensor_tensor(out=tmp[:], in0=m2[:], in1=rcp[:], op=ALU.mult)
                    nc.vector.tensor_tensor(out=tmp[:], in0=tmp[:], in1=mag[:], op=ALU.add)
                    m1 = bk.tile([128, WP], F32, tag="m1")
                    nc.vector.tensor_scalar(out=m1[:], in0=tmp[:], scalar1=0.5,
                                            scalar2=0.001, op0=ALU.mult, op1=ALU.add)
                    nn_ = bk.tile([128, WP], F32, tag="nn")
                    for kb, cj in enumerate(COS_B):
                        nc.vector.tensor_scalar_mul(out=tmp[:], in0=m1[:], scalar1=cj)
                        if kb == 0:
                            nc.vector.tensor_tensor(out=nn_[:], in0=v1, in1=tmp[:], op=ALU.is_lt)
                        else:
                            cmp = bk.tile([128, WP], F32, tag="cmp")
                            nc.vector.tensor_tensor(out=cmp[:], in0=v1, in1=tmp[:], op=ALU.is_lt)
                            nc.vector.tensor_tensor(out=nn_[:], in0=nn_[:], in1=cmp[:], op=ALU.add)
                    neg = bk.tile([128, WP], F32, tag="neg")
                    nc.vector.tensor_single_scalar(out=neg[:], in_=v0, scalar=0.0, op=ALU.is_lt)
                    nz = bk.tile([128, WP], F32, tag="nz")
                    nc.vector.tensor_single_scalar(out=nz[:], in_=nn_[:], scalar=0.5, op=ALU.is_gt)
                    bn = bk.tile([128, WP], F32, tag="bn")
                    nc.vector.tensor_scalar(out=bn[:], in0=nn_[:], scalar1=-1.0,
                                            scalar2=8.0, op0=ALU.mult, op1=ALU.add)
                    nc.vector.tensor_tensor(out=bn[:], in0=bn[:], in1=nz[:], op=ALU.mult)
                    nc.vector.tensor_tensor(out=bn[:], in0=bn[:], in1=nn_[:], op=ALU.subtract)
                    nc.vector.tensor_tensor(out=bn[:], in0=bn[:], in1=neg[:], op=ALU.mult)
                    nc.vector.tensor_tensor(out=bkt[j][:], in0=nn_[:], in1=bn[:], op=ALU.add)
                    nc.vector.tensor_single_scalar(out=tmp[:], in_=m2[:], scalar=0.01, op=ALU.is_le)
                    nc.vector.tensor_scalar_mul(out=tmp[:], in0=tmp[:], scalar1=99.0)
                    nc.vector.tensor_tensor(out=bkt[j][:], in0=bkt[j][:], in1=tmp[:], op=ALU.add)

                # ---- 8 direction steps ----
                for a in range(8):
                    dy, dx = DIRS[a]
                    for j in range(3):
                        wnd = slice(1, 513)

                        # masks
                        esh = mk.tile([128, WP], BF16, tag="esh")
                        nc.vector.tensor_single_scalar(
                            out=esh[:, wnd], in_=stf[j][:, 0, 1 + dx:513 + dx],
                            scalar=0.5, op=ALU.is_gt)
                        nw = mk.tile([128, WP], BF16, tag="nw")
                        nc.vector.tensor_single_scalar(
                            out=nw[:, wnd], in_=stf[j][:, 1, wnd], scalar=0.5, op=ALU.is_lt)
                        ma = mk.tile([128, WP], BF16, tag="ma")
                        nc.vector.tensor_single_scalar(
                            out=ma[:, wnd], in_=bkt[j][:, wnd], scalar=float(a), op=ALU.is_equal)
                        man = mk.tile([128, WP], BF16, tag="man")
                        nc.vector.tensor_tensor(out=man[:, wnd], in0=ma[:, wnd],
                                                in1=nw[:, wnd], op=ALU.mult)
                        pe_e = pmp.tile([128, 512], F32, tag="pm")
                        nc.tensor.matmul(pe_e[:], smb[:, sh_idx(dy), :], esh[:, wnd],
                                         start=True, stop=True)
                        nc.vector.tensor_tensor(out=sa[j][:, wnd], in0=man[:, wnd],
                                                in1=pe_e[:], op=ALU.mult)
                        # coef = 1 - sa - shift_opp(sa)
                        pe_s = pmp.tile([128, 512], F32, tag="pm")
                        nc.tensor.matmul(pe_s[:], smb[:, sh_idx(-dy), :],
                                         sa[j][:, 1 - dx:513 - dx], start=True, stop=True)
                        q = mk.tile([128, WP], BF16, tag="q")
                        nc.vector.tensor_scalar(out=q[:, wnd], in0=sa[j][:, wnd],
                                                scalar1=-1.0, scalar2=1.0,
                                                op0=ALU.mult, op1=ALU.add)
                        coef = mk.tile([128, WP], BF16, tag="coef")
                        nc.vector.tensor_tensor(out=coef[:, wnd], in0=q[:, wnd],
                                                in1=pe_s[:], op=ALU.subtract)

                        sab = sa[j][:].unsqueeze(1)
                        cob = coef[:].unsqueeze(1)

                        # payload groups of 3 channels (16 = 3*5 + 1; last group: p15 + w0 + w1)
                        groups = [list(range(g, min(g + 3, 15))) for g in range(0, 15, 3)]
                        for gi, chs in enumerate(groups + [[15]]):
                            n3 = len(chs)
                            last = (gi == 5)
                            c0 = chs[0]
                            t3 = big.tile([128, 3, WP], BF16, tag="t3")
                            u3 = big.tile([128, 3, WP], BF16, tag="u3")
                            nc.vector.tensor_tensor(
                                out=t3[:, 0:n3, :], in0=stp[j][:, c0:c0 + n3, :],
                                in1=sab.broadcast_to([128, n3, WP]), op=ALU.mult)
                            nc.vector.tensor_tensor(
                                out=u3[:, 0:n3, :], in0=stp[j][:, c0:c0 + n3, :],
                                in1=cob.broadcast_to([128, n3, WP]), op=ALU.mult)
                            ps = psp.tile([128, 3, 512], F32, tag="ps")
                            for ci in range(n3):
                                nc.tensor.matmul(ps[:, ci, :], smb[:, sh_idx(-dy), :],
                                                 t3[:, ci, 1 - dx:513 - dx],
                                                 start=True, stop=False)
                            for ci in range(n3):
                                nc.tensor.matmul(ps[:, ci, :], smb[:, 0, :],
                                                 u3[:, ci, wnd], start=False, stop=True)
                            nc.scalar.copy(out=stp[j][:, c0:c0 + n3, wnd], in_=ps[:, 0:n3, :])
                            if last:
                                # w0, w1 (f32) + I@sa into w0
                                tf = big.tile([128, 2, WP], F32, tag="tf")
                                uf = big.tile([128, 2, WP], F32, tag="uf")
                                saf = mk.tile([128, WP], F32, tag="saf")
                                cof = mk.tile([128, WP], F32, tag="cof")
                                nc.vector.tensor_copy(out=saf[:, wnd], in_=sa[j][:, wnd])
                                nc.vector.tensor_copy(out=cof[:, wnd], in_=coef[:, wnd])
                                nc.vector.tensor_tensor(
                                    out=tf[:], in0=stf[j][:, 0:2, :],
                                    in1=saf[:].unsqueeze(1).broadcast_to([128, 2, WP]),
                                    op=ALU.mult)
                                nc.vector.tensor_tensor(
                                    out=uf[:], in0=stf[j][:, 0:2, :],
                                    in1=cof[:].unsqueeze(1).broadcast_to([128, 2, WP]),
                                    op=ALU.mult)
                                ps2 = psp.tile([128, 3, 512], F32, tag="ps")
                                for ci in range(2):
                                    nc.tensor.matmul(ps2[:, ci, :], smf[:, sh_idx(-dy), :],
                                                     tf[:, ci, 1 - dx:513 - dx],
                                                     start=True, stop=False)
                                for ci in range(2):
                                    nc.tensor.matmul(ps2[:, ci, :], smf[:, 0, :],
                                                     uf[:, ci, wnd], start=False,
                                                     stop=(ci == 1))
                                nc.tensor.matmul(ps2[:, 0, :], smf[:, 0, :],
                                                 saf[:, wnd], start=False, stop=True)
                                nc.scalar.copy(out=stf[j][:, 0:2, wnd], in_=ps2[:, 0:2, :])

                        # velocity delta
                        tvh = big.tile([128, 2, WP], F32, tag="tvh")
                        sfb = mk.tile([128, WP], F32, tag="sfb")
                        nc.vector.tensor_scalar_mul(out=sfb[:, wnd], in0=sa[j][:, wnd],
                                                    scalar1=0.5)
                        nc.vector.tensor_tensor(
                            out=tvh[:], in0=stf[j][:, 2:4, :],
                            in1=sfb[:].unsqueeze(1).broadcast_to([128, 2, WP]), op=ALU.mult)
                        nc.vector.tensor_tensor(out=vd[j][:], in0=vd[j][:], in1=tvh[:],
                                                op=ALU.subtract)
                        psv = psp.tile([128, 3, 512], F32, tag="ps")
                        for ci in range(2):
                            nc.tensor.matmul(psv[:, ci, :], smf[:, sh_idx(-dy), :],
                                             tvh[:, ci, 1 - dx:513 - dx],
                                             start=True, stop=True)
                        nc.vector.tensor_tensor(out=vd[j][:, :, wnd], in0=vd[j][:, :, wnd],
                                                in1=psv[:, 0:2, :], op=ALU.add)

            # ---- final: vel*0.95, 3x3 conv/18 + 0.5*vel, write into stf v slots ----
            for j in range(3):
                wnd = slice(1, 513)
                vh = cv.tile([128, 2, WP], F32, tag="vh")
                nc.vector.tensor_scalar_mul(out=vh[:], in0=vd[j][:], scalar1=0.95)
                t2 = cv.tile([128, 2, WP], F32, tag="t2")
                nc.vector.tensor_tensor(out=t2[:, :, 0:513], in0=vh[:, :, 0:513],
                                        in1=vh[:, :, 1:514], op=ALU.add)
                rs = cv.tile([128, 2, WP], F32, tag="rs")
                nc.vector.tensor_tensor(out=rs[:, :, wnd], in0=t2[:, :, 0:512],
                                        in1=vh[:, :, 2:514], op=ALU.add)
                psc = psp.tile([128, 3, 512], F32, tag="ps")
                for ci in range(2):
                    nc.tensor.matmul(psc[:, ci, :], smf[:, 1, :], rs[:, ci, wnd],
                                     start=True, stop=False)
                    nc.tensor.matmul(psc[:, ci, :], smf[:, 2, :], rs[:, ci, wnd],
                                     start=False, stop=False)
                    nc.tensor.matmul(psc[:, ci, :], smf[:, 0, :], rs[:, ci, wnd],
                                     start=False, stop=True)
                o1 = cv.tile([128, 2, WP], F32, tag="o1")
                nc.vector.tensor_scalar_mul(out=o1[:, :, wnd], in0=psc[:, 0:2, :],
                                            scalar1=1.0 / 18.0)
                nc.vector.tensor_scalar_mul(out=vh[:], in0=vh[:], scalar1=0.5)
                nc.vector.tensor_tensor(out=stf[j][:, 2:4, wnd], in0=o1[:, :, wnd],
                                        in1=vh[:, :, wnd], op=ALU.add)

    nc.all_engine_barrier()
    for j in range(3):
        nc.sync.dma_start(outf[j], stf[j][HALO:HALO + UNIQ, :, 1:513]).then_inc(sem, 16)
        sv += 16
        nc.sync.dma_start(outp[j], stp[j][HALO:HALO + UNIQ, :, 1:513]).then_inc(sem, 16)
        sv += 16
    for eng in nc.engines.values():
        eng.wait_ge(sem, sv)

    _fix_sync_overflow(nc)
    return nc


def _shift_mats():
    smf = np.zeros((128, 3, 128), np.float32)
    smf[:, 0, :] = np.eye(128, dtype=np.float32)
    smf[:, 1, :] = np.eye(128, k=-1, dtype=np.float32)  # out[p] = in[p+1]
    smf[:, 2, :] = np.eye(128, k=1, dtype=np.float32)   # out[p] = in[p-1]
    return smf, smf.astype(ml_dtypes.bfloat16)


def kernel(world, velocity_field, elem_empty):
    world = np.asarray(world, dtype=np.float32)
    velocity_field = np.asarray(velocity_field, dtype=np.float32)
    if "nc" not in _CACHE:
        _CACHE["nc"] = _build_nc()
    nc = _CACHE["nc"]
    smf, smb = _shift_mats()

    in_maps = []
    for core in range(8):
        b, half = core // 2, core % 2
        start = half * 256
        inpf = np.zeros((3, 128, 4, WP), np.float32)
        inpp = np.zeros((3, 128, NP, WP), np.float32)
        for j in range(3):
            r0 = start + UNIQ * j - HALO
            lo, hi = max(0, r0), min(H, r0 + 128)
            inpf[j, lo - r0:hi - r0, 0:2, 1:513] = world[b, 0:2, lo:hi, :].transpose(1, 0, 2)
            inpf[j, lo - r0:hi - r0, 2:4, 1:513] = velocity_field[b, :, lo:hi, :].transpose(1, 0, 2)
            inpp[j, lo - r0:hi - r0, :, 1:513] = world[b, 2:NEV, lo:hi, :].transpose(1, 0, 2)
        in_maps.append({"inpf": inpf, "inpp": inpp.astype(ml_dtypes.bfloat16),
                        "smf": smf, "smb": smb})

    res = run_bass_kernel_spmd(nc, in_maps, core_ids=list(range(8)))

    out_world = np.empty((B, C, H, W), np.float32)
    out_vel = np.empty((B, 2, H, W), np.float32)
    for core in range(8):
        b, half = core // 2, core % 2
        start = half * 256
        rf = res.results[core]["outf"]
        rp = res.results[core]["outp"].astype(np.float32)
        for j in range(3):
            u0 = start + UNIQ * j
            nrows = min(UNIQ, start + 256 - u0)
            if nrows <= 0:
                continue
            out_world[b, 0:2, u0:u0 + nrows, :] = rf[j, :nrows, 0:2, :].transpose(1, 0, 2)
            out_world[b, 2:NEV, u0:u0 + nrows, :] = rp[j, :nrows, :, :].transpose(1, 0, 2)
            out_world[b, NEV:C, u0:u0 + nrows, :] = rf[j, :nrows, 2:4, :].transpose(1, 0, 2)
            out_vel[b, :, u0:u0 + nrows, :] = rf[j, :nrows, 2:4, :].transpose(1, 0, 2)
    return out_world, out_vel
